# revision 64
# baseline (speedup 1.0000x reference)
# Trainium2 Bass kernel for nn_Attention_43215960932503.
#
# Module: per-head attention over N=56*56=3136 tokens, 8 heads, B=2,
# key_dim=16, v_dim=32, with 1x1-conv+BN projections (BN folded to
# scale+bias) and a final 1x1-conv projection over all heads.
#
# Sharding: 16 (batch, head) pairs over 8 cores -> each core owns one
# batch and two adjacent heads; host sums the 4 partial final
# projections per batch and adds the output bias (exact: linear).
#
# Key optimizations over the fp32 baseline:
#  * all matmuls stream 1-byte/2-byte operands (bf16 = 1 PE cycle/row,
#    fp8e4+DoubleRow = 0.5) instead of fp32 (4 cycles/row)
#  * exp work is split between the ACT engine (true exp -> fp8 P) and
#    the DVE (Schraudolph bit-trick exp: y = S*a + b written as int8 is
#    the bit pattern of fp8 e4m3 exp(S); max ~4% elementwise error,
#    cancels in softmax normalization)
#  * softmax algebra: the key-side bias bk shifts every score of a
#    softmax row equally -> dropped entirely; the query-side bias bq
#    rides the q-cast as an ACT Identity per-partition bias; the
#    fp8-range weight prescale (x8 on Wq,Wk,Wv) folds into exp's scale
#    (1/64) and the rowsum ones-col (8.0), so every other PSUM->SBUF
#    projection copy is a pure dtype cast
#  * n-chunks are 448 wide so one QK m-tile output is <= 1 PSUM bank:
#    a PAIR of m-tiles shares a 2-bank PSUM tile and ONE exp
#    instruction (halves the elementwise fixed costs), the score
#    rotation is 3 pair-slots = 6 tiles in flight (vs 3 of 784), and
#    po is double-buffered so blocks overlap their normalize chains
#  * the rowsum (an 8.0-col matmul lane in AV) is reciprocated
#    directly from PSUM (f16), broadcast by a ones matmul on PE, and
#    relu*mul'd into bf16 z feeding the output projection
#  * heavy software pipelining via emission order (engines execute
#    in-order): AV matmuls and the normalize chain are deferred into
#    later pairs, chunk j's output projection is emitted inside later
#    blocks, and the phase-B projections (x cols 1568:3136) are
#    emitted inside block (0,0)'s m-loop so nothing serializes on the
#    full input DMA (x/st each arrive as two large DMAs: HWDGE
#    descriptor generation costs ~625ns per dma_start)
import numpy as np

N = 3136          # tokens = 56*56
NT = 448          # n-chunk (7 per row); one m-tile of scores = 1792B
NJ = 7            # n-chunks
MTILES = [(i * 128, 128) for i in range(24)] + [(3072, 64)]  # (offset, rows)
# exp engine per pair within a block (ACT=true exp, DVE=Schraudolph);
# tile 24 rotates separately over the 14 blocks
import os
# exp engine per pair: 7 ACT / 5 DVE on even blocks, 6/6 on odd blocks
# (ACT's true exp is cheaper per instr than DVE's Schraudolph, but DVE
# also owns the reciprocal + relu*mul per block)
PAIRS_TBL = [("act", "dve", "act", "act", "dve", "act",
              "dve", "act", "dve", "act", "dve", "act"),
             ("act", "dve", "act", "act", "dve", "act",
              "dve", "act", "dve", "act", "dve", "act")]
E24 = ("dve", "act") * 7
DEFER_RECIP = True
ALT_CYCLE = ("dve", "act", "dve")
QCAST_ALT = True
POOL_BCAST = False
PBC_ENG = "act"
DMA_ORDER = 0
if os.environ.get("KCFG"):
    _c = os.environ["KCFG"].split(",")
    E24 = {"d": ("dve",) * 14, "a": ("act",) * 14,
           "da": ("dve", "act") * 7}[_c[0]]
    DEFER_RECIP = _c[1] == "1"
    if len(_c) > 2 and _c[2] == "dh":
        ALT_CYCLE = ("dve", "act", "dve")
    if len(_c) > 2 and _c[2] == "ba":
        ALT_CYCLE = ("dve", "act")
    if len(_c) > 3 and _c[3] == "flat":
        PAIRS_TBL[1] = PAIRS_TBL[0]
    if len(_c) > 3 and _c[3] == "mix6":
        PAIRS_TBL[1] = ("act", "dve", "act", "dve", "dve", "act",
                        "dve", "act", "dve", "act", "dve", "act")
    if len(_c) > 4 and _c[4] == "qd":
        QCAST_ALT = True
    if len(_c) > 4 and _c[4] == "qq":
        QCAST_ALT = False
    if len(_c) > 5 and _c[5] == "pb":
        POOL_BCAST = True
    if len(_c) > 5 and _c[5] == "dmab":
        POOL_BCAST = "dma"
    if len(_c) > 6:
        PBC_ENG = {"pa": "act", "pd": "dve", "px": None}[_c[6]]
    if len(_c) > 7:
        DMA_ORDER = int(_c[7])
    if len(_c) > 3 and _c[3] == "a8":
        PAIRS_TBL[0] = PAIRS_TBL[1] = (
            "act", "dve", "act", "act", "dve", "act",
            "dve", "act", "act", "act", "dve", "act")
    if len(_c) > 3 and _c[3] == "a9":
        PAIRS_TBL[0] = PAIRS_TBL[1] = (
            "act", "dve", "act", "act", "dve", "act",
            "act", "act", "dve", "act", "act", "act")
VG_A = [range(0, 4), range(4, 8), range(8, 12)]
VG_B = [range(12, 16), range(16, 20), range(20, 24), range(24, 25)]
SCHRAU_A = 8.0 / np.log(2.0)        # fp8e4m3 Schraudolph slope
SCHRAU_B = 56.1                     # 7*8 + 0.5 (trunc->round) - 0.4 (minimax)
WSCALE = 8.0                        # host weight prescale for fp8 range

_CACHE = {}


def _build():
    import concourse.bass as bass
    import concourse.mybir as mybir
    import concourse.tile as tile
    from contextlib import ExitStack

    f32 = mybir.dt.float32
    bf16 = mybir.dt.bfloat16
    f16 = mybir.dt.float16
    f8 = mybir.dt.float8e4
    i8 = mybir.dt.int8
    EXP = mybir.ActivationFunctionType.Exp
    IDENT = mybir.ActivationFunctionType.Identity
    COPY = mybir.ActivationFunctionType.Copy
    RELU = mybir.ActivationFunctionType.Relu
    MAX = mybir.AluOpType.max
    MULT = mybir.AluOpType.mult
    ADD = mybir.AluOpType.add
    DR = mybir.MatmulPerfMode.DoubleRow

    nc = bass.Bass()
    x = nc.dram_tensor("x", (256, N), f8, kind="ExternalInput")
    st = nc.dram_tensor("st", (256, N), f8, kind="ExternalInput")
    wqT = nc.dram_tensor("wqT", (128, 2, 2, 16), f8, kind="ExternalInput")
    wkT = nc.dram_tensor("wkT", (128, 2, 2, 16), f8, kind="ExternalInput")
    wvT = nc.dram_tensor("wvT", (128, 2, 64), f8, kind="ExternalInput")
    wpT = nc.dram_tensor("wpT", (32, 2, 256), bf16, kind="ExternalInput")
    bqr = nc.dram_tensor("bqr", (16, 2), f32, kind="ExternalInput")
    bv = nc.dram_tensor("bv", (1, 64), f8, kind="ExternalInput")
    y = nc.dram_tensor("y", (256, N), f32, kind="ExternalOutput")
    scr = nc.dram_tensor("scr", (2, NT), f16, kind="Internal")
    scr_d = scr[:]

    with ExitStack() as ctx:
        tc = ctx.enter_context(tile.TileContext(nc))
        sb = ctx.enter_context(tc.tile_pool(name="sb", bufs=1))
        ptp = ctx.enter_context(tc.tile_pool(name="ptp", bufs=7))
        zp = ctx.enter_context(tc.tile_pool(name="zp", bufs=2))
        yp = ctx.enter_context(tc.tile_pool(name="yp", bufs=2))
        rp = ctx.enter_context(tc.tile_pool(name="rp", bufs=2))
        psa = ctx.enter_context(tc.tile_pool(name="psa", bufs=3, space="PSUM"))
        pop = ctx.enter_context(tc.tile_pool(name="pop", bufs=2, space="PSUM"))

        # ---- persistent SBUF tiles ----
        x_sb = sb.tile([128, 2, N], f8)       # x, dim1 = channel chunk
        st_sb = sb.tile([128, 2, N], f8)
        q_sb = sb.tile([16, 2, N], bf16)      # raw 8*q + 8*bq per head
        k_sb = sb.tile([16, 2, N], bf16)
        # [pair][head][member][v|8|pad]: member stride must be a power of
        # two for the DoubleRow ldweights ISA encoding, hence pad to 64
        vT8_sb = sb.tile([128, 13, 2, 2, 64], f8)
        wq_sb = sb.tile([128, 2, 2, 16], f8)   # [p][head][cc][kd]
        wk_sb = sb.tile([128, 2, 2, 16], f8)
        wv_sb = sb.tile([128, 2, 64], f8)
        wp_sb = sb.tile([32, 2, 256], bf16)
        bqr_sb = sb.tile([16, 2, 1], f32)     # 8*bq, q-cast bias per kd
        bv_sb = sb.tile([1, 64], f8)
        ones_sb = sb.tile([1, 128], f8)       # v-bias stationary ones row
        ones33 = sb.tile([33, 32], f16)       # rowsum-recip broadcast

        # ---- input DMAs: HWDGE descriptor-gen serializes at ~625ns per
        # dma_start, so both 128-channel chunks ride one descriptor and
        # x/st are cut into two column spans each, ordered so the first
        # k projections and the chunk-0/1 q projections start early
        XSPLIT = 1568

        def dmain(sb_t, dram_t, s, e):
            src = dram_t[:].rearrange("(c p) n -> p c n", c=2)
            nc.sync.dma_start(sb_t[:, :, s:e], src[:, :, s:e])

        if DMA_ORDER == 0:
            dmain(x_sb, x, 0, XSPLIT)
            nc.sync.dma_start(wk_sb[:], wkT[:])
            nc.sync.dma_start(wv_sb[:], wvT[:])
            dmain(st_sb, st, 0, XSPLIT)
            nc.sync.dma_start(wq_sb[:], wqT[:])
            nc.sync.dma_start(bqr_sb[:, :, 0], bqr[:])
            nc.sync.dma_start(bv_sb[:], bv[:])
            dmain(x_sb, x, XSPLIT, N)
            nc.sync.dma_start(wp_sb[:], wpT[:])
            dmain(st_sb, st, XSPLIT, N)
        elif DMA_ORDER == 1:
            # tiny head chunks first so the k0/q0 projections and the
            # first QK pairs light up ~1us earlier
            dmain(x_sb, x, 0, NT)
            nc.sync.dma_start(wk_sb[:], wkT[:])
            dmain(st_sb, st, 0, NT)
            nc.sync.dma_start(wq_sb[:], wqT[:])
            nc.sync.dma_start(wv_sb[:], wvT[:])
            nc.sync.dma_start(bqr_sb[:, :, 0], bqr[:])
            nc.sync.dma_start(bv_sb[:], bv[:])
            dmain(x_sb, x, NT, XSPLIT)
            dmain(st_sb, st, NT, XSPLIT)
            dmain(x_sb, x, XSPLIT, N)
            nc.sync.dma_start(wp_sb[:], wpT[:])
            dmain(st_sb, st, XSPLIT, N)
        else:
            dmain(x_sb, x, 0, NT)
            nc.sync.dma_start(wk_sb[:], wkT[:])
            nc.sync.dma_start(wv_sb[:], wvT[:])
            dmain(st_sb, st, 0, NT)
            nc.sync.dma_start(wq_sb[:], wqT[:])
            dmain(x_sb, x, NT, XSPLIT)
            nc.sync.dma_start(bqr_sb[:, :, 0], bqr[:])
            nc.sync.dma_start(bv_sb[:], bv[:])
            dmain(st_sb, st, NT, XSPLIT)
            dmain(x_sb, x, XSPLIT, N)
            nc.sync.dma_start(wp_sb[:], wpT[:])
            dmain(st_sb, st, XSPLIT, N)

        # memsets on the otherwise-idle Pool engine (SBUF only: GpSimd
        # has no PSUM port); only column 32 of each 64-wide vT8 slot
        # (the x8 rowsum ones-col) is read outside the 0:32 range the
        # v-casts fully overwrite.  partition_broadcast lives in the
        # 'attn' GpSimd library, not the default 'standard' one.
        if POOL_BCAST:
            from concourse import library_config
            nc.gpsimd.load_library(library_config.attn)
        nc.gpsimd.memset(ones_sb[:], 1.0)
        nc.gpsimd.memset(ones33[:], 1.0)
        nc.gpsimd.memset(vT8_sb[:, :, :, :, 32:33], WSCALE)

        def cast_copy(eng, out_ap, in_ap):
            # pure-dtype-cast PSUM->SBUF copy (ACT or DVE only)
            if eng == "act":
                nc.scalar.activation(out=out_ap, in_=in_ap, func=COPY)
            else:
                nc.vector.tensor_copy(out_ap, in_ap)

        ncopy = [0]

        def alt():
            ncopy[0] += 1
            return ALT_CYCLE[ncopy[0] % len(ALT_CYCLE)]

        def emit_kproj(t):
            # both heads' k projections share one PSUM pair-tile
            # (members 512-padded so each matmul output stays inside a
            # bank) so the PSUM->SBUF cast is a single instruction
            s = t * NT
            pk = psa.tile([16, 2, 512], f32, tag="ps", bufs=3, name="pk")
            for h in range(2):
                nc.tensor.matmul(
                    pk[:, h, 0:NT], wk_sb[:, h], x_sb[:, :, s:s + NT],
                    perf_mode=DR, start=True, stop=True)
            cast_copy(alt(), k_sb[:, :, s:s + NT], pk[:, :, 0:NT])

        def emit_vgroup(g):
            # up to 4 m-tiles' v projections share one PSUM tile and
            # one cast; vT8 wants (pair, member, head) iteration so the
            # out AP permutes the head/member dims
            gl = list(g)
            pv = psa.tile([128, len(gl), 64], f32, tag="ps", bufs=3,
                          name="pv")
            for gi, i in enumerate(gl):
                mo, mi = MTILES[i]
                for c in range(2):
                    nc.tensor.matmul(
                        pv[0:mi, gi, :], x_sb[:, c, mo:mo + mi],
                        wv_sb[:, c, :], start=(c == 0), stop=False)
                nc.tensor.matmul(
                    pv[0:mi, gi, :], ones_sb[:, 0:mi], bv_sb[:],
                    start=False, stop=True)
            i0 = gl[0]
            if len(gl) == 4:
                # ISA mem patterns allow at most 3 free dims, so cast
                # one PAIR (2 m-tiles x 2 heads x 32) per instruction
                for pr in range(2):
                    in_ap = pv[:, 2 * pr:2 * pr + 2, :].rearrange(
                        "p m (a b) -> p m a b", a=2)
                    out_ap = vT8_sb[:, i0 // 2 + pr, :, :, 0:32]
                    out_ap = out_ap.transpose([0, 2, 1, 3])
                    cast_copy(alt(), out_ap, in_ap)
            else:
                mi = MTILES[i0][1]
                in_ap = pv[0:mi, 0, :].rearrange("p (a b) -> p a b", a=2)
                out_ap = vT8_sb[0:mi, i0 // 2, :, i0 % 2, 0:32]
                cast_copy(alt(), out_ap, in_ap)

        def emit_qproj(t):
            # both heads' q projections share one PSUM pair-tile; the
            # PSUM->SBUF cast rides ACT's Identity with the 8*bq bias
            # folded in per kd partition (Identity and Exp live in the
            # same activation table, so no table reloads)
            s = t * NT
            pq = psa.tile([16, 2, 512], f32, tag="ps", bufs=3, name="pq")
            for h in range(2):
                nc.tensor.matmul(
                    pq[:, h, 0:NT], wq_sb[:, h], st_sb[:, :, s:s + NT],
                    perf_mode=DR, start=True, stop=True)
            for h in range(2):
                if QCAST_ALT and (t + h) % 2:
                    nc.vector.tensor_scalar(
                        out=q_sb[:, h, s:s + NT], in0=pq[:, h, 0:NT],
                        scalar1=bqr_sb[:, h, :], scalar2=None, op0=ADD)
                else:
                    nc.scalar.activation(
                        out=q_sb[:, h, s:s + NT], in_=pq[:, h, 0:NT],
                        func=IDENT, bias=bqr_sb[:, h, :])

        # phase A: everything x/st cols 0:1568 enables
        for t in (0, 1, 2):
            emit_kproj(t)
        for g in VG_A:
            emit_vgroup(g)
        emit_qproj(0)
        emit_qproj(1)

        # ---- attention blocks: 7 n-chunks x 2 heads ----
        zs = {}
        proj_y = {}

        def emit_proj(j, oc):
            py = psa.tile([128, NT], f32, tag="ps", bufs=3, name="py")
            for h in range(2):
                nc.tensor.matmul(
                    py[:], wp_sb[:, h, 128 * oc:128 * (oc + 1)],
                    zs[(j, h)][:], start=(h == 0), stop=(h == 1))
            if oc == 0:
                proj_y[j] = yp.tile([128, 2, NT], f32, tag="y", name="ysb")
            y_sb = proj_y[j]
            cast_copy(alt(), y_sb[:, oc, :], py[:])
            if oc == 1:
                jc = j * NT
                dst = y[:, jc:jc + NT].rearrange("(c p) n -> p c n", c=2)
                nc.sync.dma_start(dst, y_sb[:])

        norm_tail = {}

        for j in range(NJ):
            for h in range(2):
                bi = 2 * j + h
                jc = j * NT
                po = pop.tile([33, NT], f32, tag="po", bufs=2, name="po")

                def qk(i, dst):
                    mo, mi = MTILES[i]
                    nc.tensor.matmul(
                        dst[0:mi, 0:NT], k_sb[:, h, mo:mo + mi],
                        q_sb[:, h, jc:jc + NT], start=True, stop=True)

                def expp(eng, ps_ap, out_ap):
                    # one exp instruction covers a whole pair (or the
                    # unpaired tile 24)
                    if eng == "act":
                        nc.scalar.activation(
                            out=out_ap, in_=ps_ap, func=EXP,
                            scale=1.0 / (WSCALE * WSCALE))
                    else:
                        nc.vector.tensor_scalar(
                            out=out_ap.bitcast(i8), in0=ps_ap,
                            scalar1=SCHRAU_A / (WSCALE * WSCALE),
                            scalar2=SCHRAU_B, op0=MULT, op1=ADD)

                def av(p, pt, start=False, stop=False):
                    nc.tensor.matmul(
                        po[:, 0:NT], vT8_sb[:, p, h, :, 0:33],
                        pt[:, :, 0:NT], perf_mode=DR,
                        start=start, stop=stop)

                def av24(pt24, start=False, stop=False):
                    m24 = MTILES[24][1]
                    nc.tensor.matmul(
                        po[:, 0:NT], vT8_sb[0:m24, 12, h, 0, 0:33],
                        pt24[0:m24, 0, 0:NT], start=start, stop=stop)

                def exp24(e24, pt24):
                    m24 = MTILES[24][1]
                    ps24 = psa.tile([128, 2, 512], f32, tag="ps",
                                    bufs=3, name="ps24")
                    qk(24, ps24[:, 0, :])
                    expp(e24, ps24[0:m24, 0, 0:NT], pt24[0:m24, 0, :])

                # mid-loop emission hooks: the previous block's
                # reciprocal (after t24's exp so DVE isn't head-of-line
                # blocked on the AV tail), its normalize tail, phase-B
                # projections (block (0,0)), and the previous chunk's
                # output projection split per half
                hooks = {0: lambda: [f() for f in norm_tail.pop("r", [])],
                         1: lambda: [f() for f in norm_tail.pop("n", [])]}
                if bi == 0:
                    hooks[3] = lambda: [emit_kproj(3), emit_kproj(4)]
                    hooks[5] = lambda: [emit_kproj(5), emit_kproj(6),
                                        emit_vgroup(VG_B[0])]
                    hooks[7] = lambda: [emit_vgroup(VG_B[1]),
                                        emit_vgroup(VG_B[2])]
                    hooks[9] = lambda: [emit_vgroup(VG_B[3])]
                if j > 0 and h == 0:
                    hooks[5] = lambda: emit_proj(j - 1, 0)
                    hooks[9] = lambda: emit_proj(j - 1, 1)

                # the unpaired tile 24 runs FIRST (except (0,0), whose
                # v projection lands mid-loop) so no serial exp24->AV24
                # chain dangles at the block boundary
                defer = 2 if bi == 13 else 4
                t24_first = bi != 0
                if t24_first:
                    pt24 = ptp.tile([128, 2, NT], f8, tag="pt", name="pt24")
                    exp24(E24[bi], pt24)
                pend = []
                for p in range(12):
                    if p in hooks:
                        hooks[p]()
                    ps = psa.tile([128, 2, 512], f32, tag="ps", bufs=3,
                                  name="ps")
                    pt = ptp.tile([128, 2, NT], f8, tag="pt", name="pt")
                    qk(2 * p, ps[:, 0, :])
                    qk(2 * p + 1, ps[:, 1, :])
                    expp(PAIRS_TBL[bi % 2][p], ps[:, :, 0:NT], pt[:])
                    pend.append(pt)
                    if len(pend) > defer:
                        if p == defer and t24_first:
                            av24(pt24, start=True)
                        av(p - defer, pend.pop(0),
                           start=(p == defer and not t24_first))
                for pi, pt_ in enumerate(pend):
                    last = pi == len(pend) - 1
                    av(12 - len(pend) + pi, pt_, stop=last and t24_first)
                if not t24_first:
                    pt24 = ptp.tile([128, 2, NT], f8, tag="pt", name="pt24")
                    exp24(E24[bi], pt24)
                    av24(pt24, stop=True)

                # normalize: f16 reciprocal of the PSUM rowsum row (DVE),
                # broadcast across the 32 v-dim partitions by the
                # otherwise-idle GpSimd engine (SBUF->SBUF; a DVE op may
                # read only ONE input from PSUM so the broadcast must not
                # land in PSUM), then one relu*mul into bf16 z; deferred
                # into the next block's stream (hooks p=0/p=1) so no
                # engine stalls at the boundary.  The last block runs
                # everything inline so the final projection overlaps.
                z = zp.tile([32, NT], bf16, tag="z", name="z")
                zs[(j, h)] = z
                r1 = rp.tile([33, NT], f16, tag="rc", name="r1")

                def rchain(po=po, r1=r1):
                    with nc.allow_low_precision(
                            reason="f16 1/rowsum: 5e-4 on a softmax scale"):
                        nc.vector.reciprocal(r1[32:33, :], po[32:33, 0:NT])

                def ntail(po=po, z=z, r1=r1):
                    rbc = rp.tile([32, NT], f16, tag="rbc", name="rbc")
                    if POOL_BCAST == "dma":
                        nc.sync.dma_start(scr_d[bi % 2], r1[32:33, :])
                        src = scr_d[bi % 2].unsqueeze(0).partition_broadcast(32)
                        nc.sync.dma_start(rbc[:], src)
                    elif POOL_BCAST:
                        nc.gpsimd.partition_broadcast(rbc[:], r1[32:33, :])
                    else:
                        pbc = psa.tile([32, NT], f32, tag="ps", bufs=3,
                                       name="pbc")
                        nc.tensor.matmul(
                            pbc[:], ones33[32:33, 0:32], r1[32:33, :],
                            start=True, stop=True)
                        cast_copy(PBC_ENG or alt(), rbc[:], pbc[:])
                    nc.vector.scalar_tensor_tensor(
                        out=z[:], in0=po[0:32, 0:NT],
                        scalar=0.0, in1=rbc[:], op0=MAX, op1=MULT)

                if bi == 13:
                    rchain()
                    ntail()
                    emit_proj(6, 0)
                    emit_proj(6, 1)
                else:
                    if DEFER_RECIP:
                        norm_tail["r"] = norm_tail.get("r", []) + [rchain]
                    else:
                        rchain()
                    norm_tail["n"] = norm_tail.get("n", []) + [ntail]
                # q projection for a later chunk sits after the
                # normalize chain (its PE matmuls land behind the AV
                # flush, exactly when ACT drains its exp backlog)
                if h == 0 and j < 5:
                    emit_qproj(j + 2)
    return nc


def _prep_in_maps(x, singlex, Wq, sq, bq, Wk, sk, bk, Wv, sv, bv, Wp, sp, bp):
    import ml_dtypes
    bf = ml_dtypes.bfloat16
    f8 = ml_dtypes.float8_e4m3
    xf = np.ascontiguousarray(x.reshape(2, 256, N), dtype=f8)
    sf = np.ascontiguousarray(singlex.reshape(2, 256, N), dtype=f8)
    Wq_s = WSCALE * sq[:, None] * Wq
    Wk_s = WSCALE * sk[:, None] * Wk
    Wv_s = WSCALE * sv[:, None] * Wv
    Wp_s = sp[:, None] * Wp
    in_maps = []
    for c in range(8):
        b, hp = c // 4, c % 4
        g0, g1 = 2 * hp, 2 * hp + 1
        # (128, 2, 2, rows): [p, h, cc, r] = W_s[rows*g_h + r, 128 cc + p]
        def wmap(W, rows):
            out = np.empty((128, 2, 2, rows), dtype=np.float32)
            for hh, g in enumerate((g0, g1)):
                blk = W[rows * g:rows * g + rows]      # (rows, 256)
                out[:, hh, 0, :] = blk[:, 0:128].T
                out[:, hh, 1, :] = blk[:, 128:256].T
            return out

        # wv keeps [p, cc, dv] (used by plain per-chunk matmuls)
        def wvmap(W, rows):
            out = np.empty((128, 2, 2 * rows), dtype=np.float32)
            for hh, g in enumerate((g0, g1)):
                blk = W[rows * g:rows * g + rows]
                out[:, 0, rows * hh:rows * hh + rows] = blk[:, 0:128].T
                out[:, 1, rows * hh:rows * hh + rows] = blk[:, 128:256].T
            return out
        in_maps.append({
            "x": xf[b],
            "st": sf[b],
            "wqT": np.ascontiguousarray(wmap(Wq_s, 16), dtype=f8),
            "wkT": np.ascontiguousarray(wmap(Wk_s, 16), dtype=f8),
            "wvT": np.ascontiguousarray(wvmap(Wv_s, 32), dtype=f8),
            "wpT": np.ascontiguousarray(
                np.stack([Wp_s[:, 32 * g0:32 * g0 + 32].T,
                          Wp_s[:, 32 * g1:32 * g1 + 32].T], 1), dtype=bf),
            "bqr": np.ascontiguousarray(
                np.stack([bq[16 * g0:16 * g0 + 16],
                          bq[16 * g1:16 * g1 + 16]], 1) * WSCALE,
                dtype=np.float32),
            "bv": np.ascontiguousarray(
                np.concatenate([WSCALE * bv[32 * g0:32 * g0 + 32],
                                WSCALE * bv[32 * g1:32 * g1 + 32]])[None, :],
                dtype=f8),
        })
    return in_maps


def _fix_bir(bir_json):
    # This toolchain's walrus accepts only ONE sync-wait per instruction
    # on several instruction structs (Matmult/LDWEIGHTS, Drain, ...).
    # Engines execute in order, so any excess waits can be hoisted onto
    # inserted same-engine NoOps immediately before the instruction.
    import json as _json
    j = _json.loads(bir_json)
    cnt = [0]

    def fix_block(bk):
        out = []
        for ins in bk.get("instructions", []):
            si = ins.get("sync_info")
            if si and si.get("on_wait") and len(si["on_wait"]) > 1:
                waits = si["on_wait"]
                for w in waits[:-1]:
                    cnt[0] += 1
                    out.append({
                        "debug": ins.get("debug"), "engine": ins["engine"],
                        "ins": [], "name": f"I-wfix-{cnt[0]}",
                        "opcode": "NoOp", "outs": [],
                        "sync_info": {"on_update": [], "on_wait": [w]}})
                si["on_wait"] = [waits[-1]]
            out.append(ins)
        bk["instructions"] = out
        for sbk in bk.get("blocks", []):
            fix_block(sbk)

    for f in j["functions"]:
        for bk in f["blocks"]:
            fix_block(bk)
    return _json.dumps(j).encode()


def _patch_compiler():
    if _CACHE.get("patched"):
        return
    import concourse.bass_utils as bu
    import concourse.bass2jax as b2j
    orig = bu.compile_bir_kernel

    def patched(bir_json, tmpdir, neff_name="file.neff"):
        return orig(_fix_bir(bir_json), tmpdir, neff_name)

    bu.compile_bir_kernel = patched
    if getattr(b2j, "compile_bir_kernel", None) is orig:
        b2j.compile_bir_kernel = patched
    _CACHE["patched"] = True


def run(trace=False, **inputs):
    from concourse.bass_utils import run_bass_kernel_spmd

    _patch_compiler()
    inputs = {k: np.asarray(v) for k, v in inputs.items()}
    if "nc" not in _CACHE:
        _CACHE["nc"] = _build()
    in_maps = _prep_in_maps(**inputs)
    res = run_bass_kernel_spmd(
        _CACHE["nc"], in_maps, core_ids=list(range(8)), trace=trace)
    bp = inputs["bp"].astype(np.float32)
    out = np.zeros((2, 256, N), dtype=np.float32)
    for c in range(8):
        out[c // 4] += res.results[c]["y"]
    out += bp[None, :, None]
    return out.reshape(2, 256, 56, 56), res


def kernel(**inputs):
    return run(**inputs)[0]
